# revision 1
# baseline (speedup 1.0000x reference)
"""Trainium2 Bass kernel for nn_DictNet loss (8-core SPMD).

Math restructuring
------------------
reference(D, x, C, mask, y, groups) decomposes as:

  Cn    = C / ||C||                      (tiny, host)
  L     = einsum('nmk,k->nm', D, Cn)     (memory-bound: 738 MB of D)
  y_hat = x - L @ x
  d     = pairwise distance matrix of y_hat rows   [N, N]
  loss  = sparsity(Cn) + sum_c u_c d u_c^T - (1/(S^2*beta)) * sum_g h_g d h_g^T

where u[c, n] = (mask_n & y_n == c) / cnt_c  (hl2 per-class weights) and
h[g, n] = multiplicity of node n in groups[g]  (hl1 group histograms).
Both weight matrices are tiny and precomputed on the host; the device never
touches an index. Per-core partial sums are combined on the host.

Sharding: D rows (node axis) split across 8 cores; y_hat rows AllGathered
(transposed) so every core can form distance tiles for its own rows.
Symmetry: d and the weights are symmetric, so each core only processes
JBLK = CORES/2 + 1 rotated column blocks (global block (pid+j) % CORES);
off-diagonal blocks are double-counted via host-scaled weights, and the
j = CORES/2 block is zero-weighted on the upper half of the cores.
"""

import math

import numpy as np

import concourse.bass as bass
import concourse.mybir as mybir
import concourse.tile as tile
from concourse import bacc
from concourse.bass_utils import run_bass_kernel_spmd
from concourse.masks import make_identity

FP32 = mybir.dt.float32
FP32R = mybir.dt.float32r
AF = mybir.ActivationFunctionType
OP = mybir.AluOpType

FULL_CFG = dict(N=4096, F=512, K=11, G=128, NCLS=7, CORES=8)


def _derived(cfg):
    N, F, K, G, NCLS, CORES = (
        cfg["N"], cfg["F"], cfg["K"], cfg["G"], cfg["NCLS"], cfg["CORES"])
    R = N // CORES              # rows per core
    assert R % 128 == 0 and N % 512 == 0 and F % 128 == 0
    NRC = R // 128              # 128-row chunks per core
    NMC = N // 128              # 128-col chunks (m axis)
    MGRP = 1024                 # m columns per D tile
    NGRP = N // MGRP            # D tile groups along m
    NFC = F // 128              # feature chunks
    JBLK = CORES // 2 + 1       # rotated col blocks each core processes
    return dict(N=N, F=F, K=K, G=G, NCLS=NCLS, CORES=CORES, R=R, NRC=NRC,
                NMC=NMC, MGRP=MGRP, NGRP=NGRP, NFC=NFC, JBLK=JBLK)


def build(cfg, reps=1, stage="full"):
    """Build the SPMD kernel (one NEFF, runs on all cores).

    reps > 1 repeats the whole computation serially (timing probe).
    stage: "dma" = D loads only, "axpy" = + AXPY, "A" = phases A+B,
    "AG"/"simAG" = + collective (simAG fakes it), "sim" = full with faked
    collective (for TimelineSim), "full" = everything.
    """
    c = _derived(cfg)
    N, F, K, G, NCLS = c["N"], c["F"], c["K"], c["G"], c["NCLS"]
    CORES, R, NRC, NMC = c["CORES"], c["R"], c["NRC"], c["NMC"]
    MGRP, NGRP, NFC, JBLK = c["MGRP"], c["NGRP"], c["NFC"], c["JBLK"]
    SUBS = MGRP // 128

    nc = bacc.Bacc("TRN2", target_bir_lowering=False, debug=False,
                   num_devices=CORES)

    # ---- I/O ----
    Dsh = nc.dram_tensor("Dsh", [R, N, K], FP32, kind="ExternalInput")
    x_in = nc.dram_tensor("x_in", [N, F], FP32R, kind="ExternalInput")
    x_own = nc.dram_tensor("x_own", [R, F], FP32, kind="ExternalInput")
    cnb_in = nc.dram_tensor("cnb", [128, K], FP32, kind="ExternalInput")
    uT_in = nc.dram_tensor("uT_sh", [R, NCLS], FP32R, kind="ExternalInput")
    hT_in = nc.dram_tensor("hT_sh", [R, G], FP32R, kind="ExternalInput")
    u_in = nc.dram_tensor("u_rot", [NCLS, JBLK, R], FP32, kind="ExternalInput")
    h_in = nc.dram_tensor("h_rot", [G, JBLK, R], FP32, kind="ExternalInput")
    dmask_in = nc.dram_tensor("dmask", [128, NRC, R], FP32, kind="ExternalInput")
    out_u = nc.dram_tensor("out_u", [NCLS, JBLK], FP32, kind="ExternalOutput")
    out_h = nc.dram_tensor("out_h", [G, JBLK], FP32, kind="ExternalOutput")

    # collective bounce buffers: rows 0..F-1 = y_hat^T (own cols), row F = sn
    agin = nc.dram_tensor("agin", [F + 1, R], FP32R)
    agout = nc.dram_tensor("agout", [CORES, F + 1, R], FP32R,
                           addr_space="Shared")

    with tile.TileContext(nc) as tc:
      for rep in range(reps):
          with tc.tile_pool(name=f"persist{rep}", bufs=1) as pp:
              ident = pp.tile([128, 128], FP32)
              make_identity(nc, ident[:])
              cnb = pp.tile([128, K], FP32)
              nc.sync.dma_start(cnb[:], cnb_in[:])

              y_sb = [pp.tile([128, F], FP32, tag=f"y{rc}", name=f"y_sb{rep}_{rc}")
                      for rc in range(NRC)]
              sn_own = [pp.tile([128, 1], FP32, tag=f"sn{rc}", name=f"sn_own{rep}_{rc}")
                        for rc in range(NRC)]
              yT_own = [pp.tile([128, R], FP32R, tag=f"yT{fc}", name=f"yT_own{rep}_{fc}")
                        for fc in range(NFC)]
              acc_u = pp.tile([NCLS, JBLK], FP32)
              acc_h = pp.tile([G, JBLK], FP32)
              if stage not in ("full", "sim"):
                  nc.vector.memset(acc_u[:], 0.0)
                  nc.vector.memset(acc_h[:], 0.0)

              # ------------- Phase A: L = sum_k cn_k * D_k; y -= L @ x -------
              with (
                  tc.tile_pool(name=f"dA{rep}", bufs=2) as dpool,
                  tc.tile_pool(name=f"lA{rep}", bufs=2 * NRC + 1) as lpool,
                  tc.tile_pool(name=f"ltA{rep}", bufs=3) as ltsb_pool,
                  tc.tile_pool(name=f"xA{rep}", bufs=2) as xpool,
                  tc.tile_pool(name=f"psLT{rep}", bufs=4, space="PSUM") as psLT,
                  tc.tile_pool(name=f"psY{rep}", bufs=1, space="PSUM") as psY,
              ):
                  ypsum = [psY.tile([128, F], FP32, tag=f"yp{rc}",
                                    name=f"ypsum{rep}_{rc}") for rc in range(NRC)]
                  for grp in range(NGRP):
                      lgs = []
                      for rc in range(NRC):
                          dt = dpool.tile([128, MGRP, K], FP32, tag="D")
                          nc.sync.dma_start(
                              dt[:],
                              Dsh[rc * 128:(rc + 1) * 128,
                                  grp * MGRP:(grp + 1) * MGRP, :])
                          if stage == "dma":
                              continue
                          lg = lpool.tile([128, MGRP], FP32, tag="L",
                                          name=f"lg{rep}_{grp}_{rc}")
                          nc.vector.tensor_scalar_mul(lg[:], dt[:, :, 0],
                                                      cnb[:, 0:1])
                          for k in range(1, K):
                              nc.vector.scalar_tensor_tensor(
                                  lg[:], dt[:, :, k], cnb[:, k:k + 1], lg[:],
                                  OP.mult, OP.add)
                          lgs.append(lg)
                      if stage in ("dma", "axpy"):
                          continue
                      xg = xpool.tile([128, SUBS, F], FP32R, tag="X")
                      nc.sync.dma_start(
                          xg[:],
                          x_in[grp * MGRP:(grp + 1) * MGRP, :]
                          .rearrange("(s p) f -> p s f", p=128))
                      for sub in range(SUBS):
                          mc = grp * SUBS + sub
                          ltp = psLT.tile([128, R], FP32, tag="LT",
                                          name=f"ltp{rep}_{grp}_{sub}")
                          for rc in range(NRC):
                              nc.tensor.transpose(
                                  ltp[:, rc * 128:(rc + 1) * 128],
                                  lgs[rc][:, sub * 128:(sub + 1) * 128],
                                  ident[:])
                          lts = ltsb_pool.tile([128, R], FP32R, tag="LTS")
                          nc.scalar.copy(lts[:], ltp[:])
                          for rc in range(NRC):
                              nc.tensor.matmul(
                                  ypsum[rc][:],
                                  lhsT=lts[:, rc * 128:(rc + 1) * 128],
                                  rhs=xg[:, sub, :],
                                  start=(mc == 0), stop=(mc == NMC - 1))

                  if stage in ("dma", "axpy"):
                      nc.vector.memset(acc_u[:], 0.0)
                      nc.vector.memset(acc_h[:], 0.0)
                      nc.sync.dma_start(out_u[:], acc_u[:])
                      nc.sync.dma_start(out_h[:], acc_h[:])
                      continue

                  # ---- Phase B: y = x_own - L @ x; sn ----
                  with (
                      tc.tile_pool(name=f"xoB{rep}", bufs=1) as xo_pool,
                      tc.tile_pool(name=f"sqB{rep}", bufs=2) as sq_pool,
                  ):
                      for rc in range(NRC):
                          xo = xo_pool.tile([128, F], FP32, tag=f"xo{rc}")
                          nc.sync.dma_start(
                              xo[:], x_own[rc * 128:(rc + 1) * 128, :])
                          nc.vector.scalar_tensor_tensor(
                              y_sb[rc][:], ypsum[rc][:], -1.0, xo[:],
                              OP.mult, OP.add)
                          sq = sq_pool.tile([128, F], FP32, tag="sq")
                          nc.scalar.activation(
                              sq[:], y_sb[rc][:], AF.Square,
                              accum_out=sn_own[rc][:])

              if stage == "A":
                  nc.sync.dma_start(out_u[:], acc_u[:])
                  nc.sync.dma_start(out_h[:], acc_h[:])
                  continue

              with tc.tile_pool(name=f"psT{rep}", bufs=NFC, space="PSUM") as psT:
                  ytp = [psT.tile([128, R], FP32, tag="yTp", name=f"ytp{rep}_{f2}")
                         for f2 in range(NFC)]
                  for rc in range(NRC):
                      for fc in range(NFC):
                          nc.tensor.transpose(
                              ytp[fc][:, rc * 128:(rc + 1) * 128],
                              y_sb[rc][:, fc * 128:(fc + 1) * 128],
                              ident[:])
                  for fc in range(NFC):
                      nc.scalar.copy(yT_own[fc][:], ytp[fc][:])
                      nc.sync.dma_start(
                          agin[fc * 128:(fc + 1) * 128, :], yT_own[fc][:])
                  for rc in range(NRC):
                      nc.sync.dma_start(
                          agin[F:F + 1, rc * 128:(rc + 1) * 128]
                          .rearrange("one p -> p one"),
                          sn_own[rc][:].bitcast(FP32R))

              # ---------------- AllGather y_hat^T + sn ----------------
              if stage in ("sim", "simAG"):
                  # TimelineSim can't run collectives: stand in DMAs with
                  # equivalent traffic.
                  for r in range(CORES):
                      nc.sync.dma_start(agout[r], agin[:])
              else:
                  nc.gpsimd.collective_compute(
                      "AllGather", OP.bypass,
                      replica_groups=[list(range(CORES))],
                      ins=[agin[:]], outs=[agout[0:CORES]])

              if stage in ("AG", "simAG"):
                  nc.sync.dma_start(out_u[:], acc_u[:])
                  nc.sync.dma_start(out_h[:], acc_h[:])
                  continue

              # ---------------- Phase D: distance tiles + weighted sums -----
              # Matmuls run in fp32r (4x faster PE); the diagonal of the j=0
              # block is zeroed via dmask so fp32r noise there cannot leak in.
              sp_eng = nc.engines[mybir.EngineType.SP]
              pid = sp_eng.partition_id()
              rot = []  # SP registers holding (pid + j) % CORES for j >= 1
              for j in range(1, JBLK):
                  rj = sp_eng.alloc_register(f"rot{rep}_{j}")
                  sp_eng.reg_alu(rj, pid, j, OP.add)
                  sp_eng.reg_alu(rj, rj, CORES, OP.mod)
                  rot.append(bass.make_scalar_value(rj, min_val=0,
                                                    max_val=CORES - 1))
              with (
                  tc.tile_pool(name=f"yTD{rep}", bufs=1) as ytd_pool,
                  tc.tile_pool(name=f"wD{rep}", bufs=1) as w_pool,
                  tc.tile_pool(name=f"snD{rep}", bufs=1) as sn_pool,
                  tc.tile_pool(name=f"sqD{rep}", bufs=4) as sqd_pool,
                  tc.tile_pool(name=f"dD{rep}", bufs=4) as dd_pool,
                  tc.tile_pool(name=f"ttD{rep}", bufs=2) as tt_pool,
                  tc.tile_pool(name=f"psG{rep}", bufs=3, space="PSUM") as psG,
                  tc.tile_pool(name=f"psV{rep}", bufs=2, space="PSUM") as psV,
                  tc.tile_pool(name=f"psS{rep}", bufs=1, space="PSUM") as psS,
              ):
                  yT_rot = [ytd_pool.tile([128, JBLK - 1, R], FP32R, tag=f"yTr{fc}",
                                          name=f"yT_rot{rep}_{fc}")
                            for fc in range(NFC)]
                  for fc in range(NFC):
                      for j in range(1, JBLK):
                          nc.sync.dma_start(
                              yT_rot[fc][:, j - 1, :],
                              agout[bass.ds(rot[j - 1], 1),
                                    fc * 128:(fc + 1) * 128, :]
                              .rearrange("r f n -> f (r n)"))
                  sn_rot = sn_pool.tile([1, JBLK - 1, R], FP32R)
                  for j in range(1, JBLK):
                      nc.sync.dma_start(
                          sn_rot[:, j - 1, :],
                          agout[bass.ds(rot[j - 1], 1), F:F + 1, :]
                          .rearrange("r one n -> one (r n)"))
                  # j = 0 (own block) needs no AllGather data: local sn row
                  sn_loc = sn_pool.tile([1, R], FP32R)
                  nc.sync.dma_start(sn_loc[:], agin[F:F + 1, :])
                  ones1f = sn_pool.tile([1, 128], FP32)
                  nc.vector.memset(ones1f[:], 1.0)
                  ones1 = sn_pool.tile([1, 128], FP32R)
                  nc.vector.tensor_copy(ones1[:], ones1f[:])
                  sncol = sn_pool.tile([128, JBLK, R], FP32)
                  for j in range(JBLK):
                      snp = psS.tile([128, R], FP32, tag="snp")
                      src_row = sn_loc[:] if j == 0 else sn_rot[:, j - 1, :]
                      nc.tensor.matmul(snp[:], lhsT=ones1[:], rhs=src_row,
                                       start=True, stop=True)
                      nc.scalar.copy(sncol[:, j, :], snp[:])

                  dmask = w_pool.tile([128, NRC, R], FP32)
                  nc.sync.dma_start(dmask[:], dmask_in[:])
                  uT_sb = w_pool.tile([128, NRC, NCLS], FP32R)
                  nc.sync.dma_start(
                      uT_sb[:], uT_in[:].rearrange("(rc p) c -> p rc c", p=128))
                  hT_sb = w_pool.tile([128, NRC, G], FP32R)
                  nc.sync.dma_start(
                      hT_sb[:], hT_in[:].rearrange("(rc p) g -> p rc g", p=128))
                  u_sb = w_pool.tile([NCLS, JBLK, R], FP32)
                  nc.sync.dma_start(u_sb[:], u_in[:])
                  h_sb = w_pool.tile([G, JBLK, R], FP32)
                  nc.sync.dma_start(h_sb[:], h_in[:])

                  tiles = [(j, rc) for j in range(JBLK) for rc in range(NRC)]
                  vu = vh = None
                  pending = None  # (j, rc, d_tile) awaiting V matmuls

                  def flush_pending():
                      nonlocal pending
                      if pending is None:
                          return
                      pj, prc, pdt = pending
                      nc.tensor.matmul(
                          vu[:], lhsT=uT_sb[:, prc, :], rhs=pdt[:],
                          start=(prc == 0), stop=(prc == NRC - 1))
                      nc.tensor.matmul(
                          vh[:], lhsT=hT_sb[:, prc, :], rhs=pdt[:],
                          start=(prc == 0), stop=(prc == NRC - 1))
                      pending = None
                      if prc == NRC - 1:
                          su = tt_pool.tile([NCLS, R], FP32, tag="su",
                                            name=f"su{rep}_{pj}")
                          nc.vector.tensor_tensor(
                              out=su[:], in0=vu[:], in1=u_sb[:, pj, :],
                              op=OP.mult)
                          nc.vector.reduce_sum(
                              acc_u[:, pj:pj + 1], su[:],
                              axis=mybir.AxisListType.X)
                          sh = tt_pool.tile([G, R], FP32, tag="sh",
                                            name=f"sh{rep}_{pj}")
                          nc.vector.tensor_tensor(
                              out=sh[:], in0=vh[:], in1=h_sb[:, pj, :],
                              op=OP.mult)
                          nc.vector.reduce_sum(
                              acc_h[:, pj:pj + 1], sh[:],
                              axis=mybir.AxisListType.X)

                  for j, rc in tiles:
                      if rc == 0:
                          new_vu = psV.tile([NCLS, R], FP32, tag="vu",
                                            name=f"vu{rep}_{j}")
                          new_vh = psV.tile([G, R], FP32, tag="vh",
                                            name=f"vh{rep}_{j}")
                      gram = psG.tile([128, R], FP32, tag="g",
                                      name=f"gram{rep}_{j}_{rc}")
                      for fc in range(NFC):
                          rhs = (yT_own[fc][:] if j == 0
                                 else yT_rot[fc][:, j - 1, :])
                          nc.tensor.matmul(
                              gram[:],
                              lhsT=yT_own[fc][:, rc * 128:(rc + 1) * 128],
                              rhs=rhs,
                              start=(fc == 0), stop=(fc == NFC - 1))
                      flush_pending()
                      if rc == 0:
                          vu, vh = new_vu, new_vh
                      sq = sqd_pool.tile([128, R], FP32, tag="sq")
                      nc.vector.scalar_tensor_tensor(
                          sq[:], gram[:], -2.0, sncol[:, j, :],
                          OP.mult, OP.add)
                      nc.vector.tensor_scalar(
                          sq[:], sq[:], sn_own[rc][:], 0.0, OP.add, OP.max)
                      dt = dd_pool.tile([128, R], FP32R, tag="d")
                      nc.scalar.activation(dt[:], sq[:], AF.Sqrt)
                      if j == 0:
                          nc.vector.tensor_tensor(
                              out=dt[:], in0=dt[:], in1=dmask[:, rc, :],
                              op=OP.mult)
                      pending = (j, rc, dt)
                  flush_pending()

                  nc.sync.dma_start(out_u[:], acc_u[:])
                  nc.sync.dma_start(out_h[:], acc_h[:])

    nc.compile()
    return nc


def host_prep(cfg, D, x, C, mask, y, groups):
    """Host-side input prep: normalize C, build weight matrices, shard."""
    c = _derived(cfg)
    N, K, G, NCLS, CORES, R = c["N"], c["K"], c["G"], c["NCLS"], c["CORES"], c["R"]
    NRC, JBLK = c["NRC"], c["JBLK"]

    C32 = np.asarray(C, np.float32)
    cn = (C32 / np.linalg.norm(C32, axis=0, keepdims=True)).astype(np.float32)
    dim = np.float32(math.sqrt(K))
    nrm = np.linalg.norm(cn, axis=0).astype(np.float32)
    sparsity = float(np.mean((dim - np.abs(cn).sum(0) / nrm) / (dim - 1.0)))

    mask_b = np.asarray(mask, bool)
    y_i = np.asarray(y, np.int64)
    cnt = np.zeros(NCLS, np.int64)
    np.add.at(cnt, y_i[mask_b], 1)
    u = np.zeros((NCLS, N), np.float32)
    sel = mask_b & (cnt[y_i] > 0)
    u[y_i[sel], np.nonzero(sel)[0]] = 1.0 / cnt[y_i[sel]]

    g_i = np.asarray(groups, np.int64)
    H = np.zeros((G, N), np.float32)
    np.add.at(H, (np.repeat(np.arange(G), g_i.shape[1]), g_i.ravel()), 1.0)

    cnb = np.tile(cn.ravel()[None, :], (128, 1)).astype(np.float32)
    uT = np.ascontiguousarray(u.T)
    hT = np.ascontiguousarray(H.T)
    x32 = np.ascontiguousarray(np.asarray(x, np.float32))
    D32 = np.asarray(D, np.float32)

    # diagonal mask for the j=0 (own) block: 0 at global col == global row
    dmask = np.ones((128, NRC, R), np.float32)
    for rc in range(NRC):
        for p in range(128):
            dmask[p, rc, rc * 128 + p] = 0.0

    in_maps = []
    for ci in range(CORES):
        sl = slice(ci * R, (ci + 1) * R)
        # rotated, symmetry-scaled weight slices: j -> global block (ci+j)%CORES
        u_rot = np.zeros((NCLS, JBLK, R), np.float32)
        h_rot = np.zeros((G, JBLK, R), np.float32)
        for j in range(JBLK):
            gb = (ci + j) % CORES
            scale = 1.0 if j == 0 else 2.0
            if j == CORES // 2 and ci >= CORES // 2:
                continue  # pair already handled by core ci - CORES//2
            u_rot[:, j, :] = u[:, gb * R:(gb + 1) * R] * scale
            h_rot[:, j, :] = H[:, gb * R:(gb + 1) * R] * scale
        in_maps.append({
            "Dsh": np.ascontiguousarray(D32[sl]),
            "x_in": x32,
            "x_own": np.ascontiguousarray(x32[sl]),
            "cnb": cnb,
            "uT_sh": np.ascontiguousarray(uT[sl]),
            "hT_sh": np.ascontiguousarray(hT[sl]),
            "u_rot": u_rot,
            "h_rot": h_rot,
            "dmask": dmask,
        })
    return in_maps, sparsity


def combine(cfg, results, sparsity, group_size):
    """loss = sparsity + hl2 + hl1/beta, from per-core partial sums."""
    beta = np.float64(cfg["G"]) / np.float64(cfg["NCLS"])
    hl2 = np.float64(0.0)
    s1 = np.float64(0.0)
    for r in results:
        hl2 += r["out_u"].astype(np.float64).sum()
        s1 += r["out_h"].astype(np.float64).sum()
    hl1 = -s1 / np.float64(group_size * group_size)
    total = np.float64(sparsity) + hl2 + hl1 / beta
    return np.float32(total)


_BUILD_CACHE = {}


def _get_nc(key, cfg):
    if key not in _BUILD_CACHE:
        _BUILD_CACHE[key] = build(cfg)
    return _BUILD_CACHE[key]


def kernel(D, x, C, mask, y, groups):
    cfg = dict(FULL_CFG)
    in_maps, sparsity = host_prep(cfg, D, x, C, mask, y, groups)
    nc = _get_nc("full", cfg)
    res = run_bass_kernel_spmd(
        nc, in_maps, core_ids=list(range(cfg["CORES"])), trace=False)
    return combine(cfg, res.results, sparsity, np.asarray(groups).shape[1])

